# revision 4
# baseline (speedup 1.0000x reference)
"""Trainium2 Bass kernel v2 for token-level contrastive loss (CLIP-style with
softmax token pooling), distributed over 8 NeuronCores.

Strategy vs v1: token axis T sharded (196 -> 200 = 8 x 25).  Host stages the
per-core shards PRE-TRANSPOSED ([t, d, b] layout) so the device loads
[d, token] tiles directly via cast-DMA -- eliminating all PE transposes and
their PSUM->SBUF copies.  Per-token norms are computed on-device from the
transposed tiles: DVE squares + indicator-matrix matmuls reduce over the d
partitions into per-group [5, 512] PSUM rows; r = 1/sqrt(n2+eps) via
Ln/Exp.  r_text folds into the exp scale (dots partition dim = b); r_vis is
broadcast via tiny PE outer-products and multiplied into visT.  S/V are
accumulated in PSUM via identity matmuls per b-tile (PSUM-resident across all
25 t), then ReduceScattered per b-tile (4 chunked collectives that overlap
compute), followed by a 2 KB AllReduce of per-core loss partials.
"""

import sys

sys.path.insert(0, "/opt/trn_rl_repo")

import numpy as np

import concourse.bass as bass
import concourse.mybir as mybir
import concourse.tile as tile
from concourse import bacc
from concourse.bass import ds, ts
from concourse.bass_utils import run_bass_kernel_spmd
from concourse.masks import make_identity

B = 512
T = 196
D = 256
NCORES = 8
TPAD = 200
TLOC = TPAD // NCORES  # 25
G = 5                  # t-slices per pipeline group
NG = TLOC // G         # 5 groups
NB = B // 128          # 4 b-tiles
NPAD = TPAD - T        # 4 zero pad slices globally
ROWS = B // NCORES     # 64 rows per core after ReduceScatter
RPC = ROWS // NB       # 16 rows per chunk per core
TEMP = 0.07

F32 = mybir.dt.float32
BF16 = mybir.dt.bfloat16
F8 = mybir.dt.float8e4
DR = mybir.MatmulPerfMode.DoubleRow

import os
_PRELOAD_SET = os.environ.get("K_NO_PRELOAD", "0") != "1"
NL_EXP_SET_ID = 6  # natural_log_exp_and_others in act_info.json order


def _build_program():
    nc = bacc.Bacc(
        "TRN2",
        target_bir_lowering=False,
        debug=False,
        num_devices=NCORES,
    )
    # transposed shards: [t, h(d-half), p(d-in-half), b]
    text_in = nc.dram_tensor("text", [TLOC, 2, 128, B], F32, kind="ExternalInput")
    vis_in = nc.dram_tensor("vis", [TLOC, 2, 128, B], F32, kind="ExternalInput")
    dmask_in = nc.dram_tensor("dmask", [ROWS, B], F32, kind="ExternalInput")
    out = nc.dram_tensor("out", [1, 1], F32, kind="ExternalOutput")

    text_v = text_in.ap().rearrange("t h p b -> p t h b")
    vis_v = vis_in.ap().rearrange("t h p b -> p t h b")

    with tile.TileContext(nc) as tc:
        with (
            tc.tile_pool(name="const", bufs=1) as cpool,
            tc.tile_pool(name="big", bufs=1) as bigpool,
            tc.tile_pool(name="sq", bufs=1) as sqpool,
            tc.tile_pool(name="rr", bufs=2) as rrpool,
            tc.tile_pool(name="et", bufs=4) as epool,
            tc.tile_pool(name="sv", bufs=2) as svpool,
            tc.tile_pool(name="fin", bufs=1) as finpool,
            tc.tile_pool(name="ps_n2", bufs=1, space="PSUM") as ps_n2,
            tc.tile_pool(name="ps_misc", bufs=1, space="PSUM") as ps_misc,
            tc.tile_pool(name="ps_dots", bufs=3, space="PSUM") as ps_dots,
            tc.tile_pool(name="ps_acc", bufs=1, space="PSUM") as ps_acc,
            tc.tile_pool(name="dram", bufs=1, space="DRAM") as dpool,
        ):
            # ---- constants ----
            ident = cpool.tile([128, 128], BF16, tag="ident")
            make_identity(nc, ident[:])

            # ident_pair[p, j, m] = 1 iff m == p, both j slices (fp8 DoubleRow)
            identp = cpool.tile([128, 2, 128], F8, tag="identp")
            nc.gpsimd.memset(identp[:], 0.0)
            nc.gpsimd.affine_select(
                out=identp[:],
                in_=identp[:],
                compare_op=mybir.AluOpType.not_equal,
                fill=1.0,
                base=0,
                pattern=[[0, 2], [1, 128]],
                channel_multiplier=-1,
            )
            # ind10[p, src, tl, j] = 1 iff j == 5*src + tl  (n2 row-select:
            # text rows 0-4, vis rows 5-9 of one [10, B] PSUM tile)
            ind10 = cpool.tile([128, 2, G, 2 * G], BF16, tag="ind10")
            nc.gpsimd.memset(ind10[:], 0.0)
            nc.gpsimd.affine_select(
                out=ind10[:],
                in_=ind10[:],
                compare_op=mybir.AluOpType.not_equal,
                fill=1.0,
                base=0,
                pattern=[[-G, 2], [-1, G], [1, 2 * G]],
                channel_multiplier=0,
            )
            # sel10[p, tl, d] = 1 iff p == 5 + tl (r_v row broadcast selector)
            sel10 = cpool.tile([2 * G, G, 128], BF16, tag="sel10")
            nc.gpsimd.memset(sel10[:], 0.0)
            nc.gpsimd.affine_select(
                out=sel10[:],
                in_=sel10[:],
                compare_op=mybir.AluOpType.not_equal,
                fill=1.0,
                base=-G,
                pattern=[[-1, G], [0, 128]],
                channel_multiplier=1,
            )
            ones64 = cpool.tile([ROWS, 1], F32, tag="ones64")
            nc.gpsimd.memset(ones64[:], 1.0)
            eps10 = cpool.tile([2 * G, 1], F32, tag="eps10")
            nc.gpsimd.memset(eps10[:], 1e-12)
            dmask = cpool.tile([ROWS, B], F32, tag="dmask")
            nc.sync.dma_start(dmask[:], dmask_in.ap())

            if _PRELOAD_SET:
                nc.scalar.add_instruction(
                    mybir.InstLoadActFuncSet(
                        name=nc.get_next_instruction_name(),
                        ins=[],
                        outs=[],
                        act_func_set_id=NL_EXP_SET_ID,
                    )
                )

            # ---- persistent SBUF ----
            textT = bigpool.tile([128, TLOC, 2, B], BF16, tag="textT")
            visT = bigpool.tile([128, TLOC, 2, B], BF16, tag="visT")
            rt_sb = bigpool.tile([128, NB, TLOC], F32, tag="rt")

            # ================= phase A: load + norms + normalize =========
            for g in range(NG):
                tg = ds(g * G, G)
                nc.gpsimd.dma_start(textT[:, tg, :, :], text_v[:, tg, :, :])
                nc.gpsimd.dma_start(visT[:, tg, :, :], vis_v[:, tg, :, :])

                n2_ps = ps_n2.tile([2 * G, B], F32, tag="n2")
                k = 0
                for src, xT in ((0, textT), (1, visT)):
                    sq = sqpool.tile([128, G, 2, B], BF16, tag=f"sq{src}")
                    nc.vector.tensor_mul(sq[:], xT[:, tg, :, :], xT[:, tg, :, :])
                    # pre-sum the two d-halves so n2 needs one matmul per t
                    sqh = sqpool.tile([128, G, B], BF16, tag=f"sqh{src}")
                    nc.vector.tensor_add(
                        sqh[:], sq[:, :, 0, :], sq[:, :, 1, :]
                    )
                    for tl in range(G):
                        nc.tensor.matmul(
                            n2_ps[:],
                            ind10[:, src, tl, :],
                            sqh[:, tl, :],
                            start=(k == 0),
                            stop=(k == 2 * G - 1),
                        )
                        k += 1
                # r rows = exp(-0.5 * ln(n2 + eps)):  [10, 512]
                scr = rrpool.tile([2 * G, B], F32, tag="scr")
                nc.scalar.activation(
                    scr[:], n2_ps[:],
                    mybir.ActivationFunctionType.Ln,
                    bias=eps10[:],
                )
                r = rrpool.tile([2 * G, B], BF16, tag="r")
                nc.scalar.activation(
                    r[:], scr[:],
                    mybir.ActivationFunctionType.Exp,
                    scale=-0.5,
                )

                # r_t -> [b-part, t] layout via PE mini-transposes
                for i in range(NB):
                    rtT_ps = ps_misc.tile([128, G], BF16, tag="rtT")
                    nc.tensor.transpose(
                        rtT_ps[:], r[0:G, ts(i, 128)], ident[0:G, 0:G]
                    )
                    nc.vector.tensor_copy(rt_sb[:, i, tg], rtT_ps[:])

                # normalize visT columns: rvb = ones (x) r_v[t]; visT *= rvb
                for tl in range(G):
                    tt = g * G + tl
                    rvb_ps = ps_misc.tile([128, B], F32, tag="rvb")
                    nc.tensor.matmul(
                        rvb_ps[:],
                        sel10[:, tl, :],
                        r[:],
                        start=True, stop=True,
                    )
                    rvb = rrpool.tile([128, B], BF16, tag="rvb_sb")
                    nc.scalar.copy(rvb[:], rvb_ps[:])
                    for h in range(2):
                        nc.vector.tensor_mul(
                            visT[:, tt, h, :], visT[:, tt, h, :], rvb[:]
                        )

            # ================= phase B: dots/exp/SV per b-tile ===========
            cc_outs = []
            for i in range(NB):
                S_ps = ps_acc.tile([128, B], F32, tag="Sps")
                V_ps = ps_acc.tile([128, B], F32, tag="Vps")
                for t in range(TLOC):
                    j = t % 2
                    if j == 0:
                        e_p = epool.tile([128, 2, B], F8, tag="e")
                        tmp_p = epool.tile([128, 2, B], F8, tag="tmp")
                    dots = ps_dots.tile([128, B], F32, tag="dots")
                    nc.tensor.matmul(
                        dots[:], textT[:, t, 0, ts(i, 128)], visT[:, t, 0, :],
                        start=True, stop=False,
                    )
                    nc.tensor.matmul(
                        dots[:], textT[:, t, 1, ts(i, 128)], visT[:, t, 1, :],
                        start=False, stop=True,
                    )
                    nc.scalar.activation(
                        e_p[:, j, :], dots[:],
                        mybir.ActivationFunctionType.Exp,
                        scale=rt_sb[:, i, ds(t, 1)],
                    )
                    nc.vector.scalar_tensor_tensor(
                        out=tmp_p[:, j, :],
                        in0=dots[:],
                        scalar=rt_sb[:, i, ds(t, 1)],
                        in1=e_p[:, j, :],
                        op0=mybir.AluOpType.mult,
                        op1=mybir.AluOpType.mult,
                    )
                    # S/V accumulation: fp8 DoubleRow per t-pair; plain fp8
                    # matmul for the odd leftover slice
                    if j == 1:
                        nc.tensor.matmul(
                            S_ps[:], identp[:], e_p[:],
                            start=(t == 1), stop=False, perf_mode=DR,
                        )
                        nc.tensor.matmul(
                            V_ps[:], identp[:], tmp_p[:],
                            start=(t == 1), stop=False, perf_mode=DR,
                        )
                    elif t == TLOC - 1:
                        nc.tensor.matmul(
                            S_ps[:], identp[:, 0, :], e_p[:, 0, :],
                            start=False, stop=True,
                        )
                        nc.tensor.matmul(
                            V_ps[:], identp[:, 0, :], tmp_p[:, 0, :],
                            start=False, stop=True,
                        )
                # flush; ReduceScatter per b-tile (4 pipelined chunks)
                S_sb = svpool.tile([128, B], F32, tag="Ssb")
                V_sb = svpool.tile([128, B], F32, tag="Vsb")
                nc.vector.tensor_copy(S_sb[:], S_ps[:])
                nc.vector.tensor_copy(V_sb[:], V_ps[:])
                cc_in1 = dpool.tile([128, 2, B], F32, tag=f"cc_in{i}")
                nc.sync.dma_start(cc_in1[:, 0, :], S_sb[:])
                nc.sync.dma_start(cc_in1[:, 1, :], V_sb[:])
                cc_out = dpool.tile([RPC, 2, B], F32, tag=f"cc_out{i}")
                nc.gpsimd.collective_compute(
                    "ReduceScatter",
                    mybir.AluOpType.add,
                    replica_groups=[list(range(NCORES))],
                    ins=[cc_in1[:].opt()],
                    outs=[cc_out[:].opt()],
                )
                cc_outs.append(cc_out)

            # ================= tail: local loss on 64 owned rows =========
            # rows layout: chunk i at partitions 16i..16i+15
            rows = finpool.tile([ROWS, 2, B], F32, tag="rows")
            for i in range(NB):
                nc.sync.dma_start(rows[ds(RPC * i, RPC), :, :], cc_outs[i][:])
            Sl = rows[:, 0, :]
            Vl = rows[:, 1, :]
            # pad correction: each global pad slice added exp(0)=1 to S
            nc.vector.tensor_scalar_add(Sl, Sl, float(-NPAD))
            # sim = V / S  (S > 0)
            scr = finpool.tile([ROWS, B], F32, tag="scr")
            nc.scalar.activation(scr[:], Sl, mybir.ActivationFunctionType.Ln)
            nc.scalar.activation(
                scr[:], scr[:], mybir.ActivationFunctionType.Exp, scale=-1.0
            )
            sim = finpool.tile([ROWS, B], F32, tag="sim")
            nc.vector.tensor_mul(sim[:], Vl, scr[:])
            # diag partial (per-partition): sum(sim * dmask) over free dim
            diag_p = finpool.tile([ROWS, 1], F32, tag="diagp")
            nc.vector.scalar_tensor_tensor(
                out=scr[:],
                in0=sim[:],
                scalar=1.0,
                in1=dmask[:],
                op0=mybir.AluOpType.mult,
                op1=mybir.AluOpType.mult,
                accum_out=diag_p[:],
            )
            # e2 = exp(sim/TEMP) with fused rowsum
            e2 = finpool.tile([ROWS, B], F32, tag="e2")
            rowsum = finpool.tile([ROWS, 1], F32, tag="rowsum")
            nc.scalar.activation(
                e2[:], sim[:],
                mybir.ActivationFunctionType.Exp,
                scale=1.0 / TEMP,
                accum_out=rowsum[:],
            )
            lse_row = finpool.tile([ROWS, 1], F32, tag="lserow")
            nc.scalar.activation(
                lse_row[:], rowsum[:], mybir.ActivationFunctionType.Ln
            )
            # partition reductions via ones-matmuls: [1,512] colsum, [1,2] scalars
            col_ps = ps_misc.tile([1, B], F32, tag="rvb")
            nc.tensor.matmul(col_ps[:], ones64[:], e2[:], start=True, stop=True)
            red_pack = finpool.tile([ROWS, 2], F32, tag="redpack")
            nc.vector.tensor_copy(red_pack[:, 0:1], lse_row[:])
            nc.vector.tensor_copy(red_pack[:, 1:2], diag_p[:])
            red_ps = ps_misc.tile([1, 2], F32, tag="rtT")
            nc.tensor.matmul(red_ps[:], ones64[:], red_pack[:], start=True, stop=True)
            # pack [1, 514] = colsum | rowlse_sum | diag_sum
            pack = finpool.tile([1, B + 2], F32, tag="pack")
            nc.vector.tensor_copy(pack[:, 0:B], col_ps[:])
            nc.vector.tensor_copy(pack[:, B : B + 2], red_ps[:])
            cc2_in = dpool.tile([1, B + 2], F32, tag="cc2_in")
            cc2_out = dpool.tile([1, B + 2], F32, tag="cc2_out", addr_space="Shared")
            nc.sync.dma_start(cc2_in[:], pack[:])
            nc.gpsimd.collective_compute(
                "AllReduce",
                mybir.AluOpType.add,
                replica_groups=[list(range(NCORES))],
                ins=[cc2_in[:].opt()],
                outs=[cc2_out[:].opt()],
            )
            packr = finpool.tile([1, B + 2], F32, tag="packr")
            nc.sync.dma_start(packr[:], cc2_out[:])
            # loss = 0.5*(rowlse_sum + sum_c ln(colsum))/B - diag/(B*TEMP)
            lse_col = finpool.tile([1, B], F32, tag="lsecol")
            nc.scalar.activation(
                lse_col[:], packr[:, 0:B], mybir.ActivationFunctionType.Ln
            )
            csum = finpool.tile([1, 1], F32, tag="csum")
            nc.vector.reduce_sum(csum[:], lse_col[:], axis=mybir.AxisListType.X)
            t_a = finpool.tile([1, 1], F32, tag="ta")
            nc.vector.tensor_add(t_a[:], packr[:, B : B + 1], csum[:])
            nc.vector.tensor_scalar_mul(t_a[:], t_a[:], 0.5 / B)
            t_b = finpool.tile([1, 1], F32, tag="tb")
            nc.vector.tensor_scalar_mul(
                t_b[:], packr[:, B + 1 : B + 2], 1.0 / (B * TEMP)
            )
            loss_t = finpool.tile([1, 1], F32, tag="loss")
            nc.vector.tensor_sub(loss_t[:], t_a[:], t_b[:])
            nc.sync.dma_start(out.ap(), loss_t[:])

    nc.compile()
    return nc


_CACHE = {}


def _get_program():
    if "nc" not in _CACHE:
        _CACHE["nc"] = _build_program()
    return _CACHE["nc"]


def _stage(text: np.ndarray, vis: np.ndarray):
    """Host-side layout staging: pad T to 200, transpose to [t, d, b],
    split d into halves, slice per core. Zero FLOPs, pure data movement."""
    tt = np.zeros((TPAD, D, B), np.float32)
    vv = np.zeros((TPAD, D, B), np.float32)
    tt[:T] = text.transpose(1, 2, 0)
    vv[:T] = vis.transpose(1, 2, 0)
    tt = tt.reshape(TPAD, 2, 128, B)
    vv = vv.reshape(TPAD, 2, 128, B)
    in_maps = []
    for k in range(NCORES):
        sl = slice(k * TLOC, (k + 1) * TLOC)
        # rows owned after per-i RS: local p -> i = p//16, r = p%16;
        # global row = 128*i + 16*k + r
        p = np.arange(ROWS)
        rows = 128 * (p // RPC) + RPC * k + (p % RPC)
        dm = np.zeros((ROWS, B), np.float32)
        dm[np.arange(ROWS), rows] = 1.0
        in_maps.append(
            {
                "text": np.ascontiguousarray(tt[sl]),
                "vis": np.ascontiguousarray(vv[sl]),
                "dmask": dm,
            }
        )
    return in_maps


def kernel(text_tokens: np.ndarray, visual_tokens: np.ndarray) -> np.ndarray:
    text = np.asarray(text_tokens, dtype=np.float32)
    vis = np.asarray(visual_tokens, dtype=np.float32)
    assert text.shape == (B, T, D) and vis.shape == (B, T, D)
    in_maps = _stage(text, vis)
    nc = _get_program()
    res = run_bass_kernel_spmd(nc, in_maps, core_ids=list(range(NCORES)))
    loss = np.float32(res.results[0]["out"].reshape(-1)[0])
    return np.asarray(loss, dtype=np.float32).reshape(())


# revision 6
# speedup vs baseline: 1.0055x; 1.0055x over previous
"""Trainium2 Bass kernel v2 for token-level contrastive loss (CLIP-style with
softmax token pooling), distributed over 8 NeuronCores.

Strategy vs v1: token axis T sharded (196 -> 200 = 8 x 25).  Host stages the
per-core shards PRE-TRANSPOSED ([t, d, b] layout) so the device loads
[d, token] tiles directly via cast-DMA -- eliminating all PE transposes and
their PSUM->SBUF copies.  Per-token norms are computed on-device from the
transposed tiles: DVE squares + indicator-matrix matmuls reduce over the d
partitions into per-group [5, 512] PSUM rows; r = 1/sqrt(n2+eps) via
Ln/Exp.  r_text folds into the exp scale (dots partition dim = b); r_vis is
broadcast via tiny PE outer-products and multiplied into visT.  S/V are
accumulated in PSUM via identity matmuls per b-tile (PSUM-resident across all
25 t), then ReduceScattered per b-tile (4 chunked collectives that overlap
compute), followed by a 2 KB AllReduce of per-core loss partials.
"""

import sys

sys.path.insert(0, "/opt/trn_rl_repo")

import numpy as np

import concourse.bass as bass
import concourse.mybir as mybir
import concourse.tile as tile
from concourse import bacc
from concourse.bass import ds, ts
from concourse.bass_utils import run_bass_kernel_spmd
from concourse.masks import make_identity

B = 512
T = 196
D = 256
NCORES = 8
TPAD = 200
TLOC = TPAD // NCORES  # 25
G = 5                  # t-slices per pipeline group
NG = TLOC // G         # 5 groups
NB = B // 128          # 4 b-tiles
NPAD = TPAD - T        # 4 zero pad slices globally
ROWS = B // NCORES     # 64 rows per core after ReduceScatter
RPC = ROWS // NB       # 16 rows per chunk per core
TEMP = 0.07

F32 = mybir.dt.float32
BF16 = mybir.dt.bfloat16
F8 = mybir.dt.float8e4
DR = mybir.MatmulPerfMode.DoubleRow

import os
_PRELOAD_SET = os.environ.get("K_NO_PRELOAD", "0") != "1"
NL_EXP_SET_ID = 6  # natural_log_exp_and_others in act_info.json order


def _build_program():
    nc = bacc.Bacc(
        "TRN2",
        target_bir_lowering=False,
        debug=False,
        num_devices=NCORES,
    )
    # transposed shards: [t, h(d-half), p(d-in-half), b]
    text_in = nc.dram_tensor("text", [TLOC, 2, 128, B], F32, kind="ExternalInput")
    vis_in = nc.dram_tensor("vis", [TLOC, 2, 128, B], F32, kind="ExternalInput")
    dmask_in = nc.dram_tensor("dmask", [ROWS, B], F32, kind="ExternalInput")
    out = nc.dram_tensor("out", [1, 1], F32, kind="ExternalOutput")

    text_v = text_in.ap().rearrange("t h p b -> p t h b")
    vis_v = vis_in.ap().rearrange("t h p b -> p t h b")

    with tile.TileContext(nc) as tc:
        with (
            tc.tile_pool(name="const", bufs=1) as cpool,
            tc.tile_pool(name="big", bufs=1) as bigpool,
            tc.tile_pool(name="sq", bufs=1) as sqpool,
            tc.tile_pool(name="rr", bufs=2) as rrpool,
            tc.tile_pool(name="et", bufs=4) as epool,
            tc.tile_pool(name="sv", bufs=2) as svpool,
            tc.tile_pool(name="fin", bufs=1) as finpool,
            tc.tile_pool(name="ps_n2", bufs=1, space="PSUM") as ps_n2,
            tc.tile_pool(name="ps_misc", bufs=1, space="PSUM") as ps_misc,
            tc.tile_pool(name="ps_dots", bufs=3, space="PSUM") as ps_dots,
            tc.tile_pool(name="ps_acc", bufs=1, space="PSUM") as ps_acc,
            tc.tile_pool(name="dram", bufs=1, space="DRAM") as dpool,
        ):
            # ---- constants ----
            ident = cpool.tile([128, 128], BF16, tag="ident")
            make_identity(nc, ident[:])

            # ident_pair[p, j, m] = 1 iff m == p, both j slices (fp8 DoubleRow)
            identp = cpool.tile([128, 2, 128], F8, tag="identp")
            nc.gpsimd.memset(identp[:], 0.0)
            nc.gpsimd.affine_select(
                out=identp[:],
                in_=identp[:],
                compare_op=mybir.AluOpType.not_equal,
                fill=1.0,
                base=0,
                pattern=[[0, 2], [1, 128]],
                channel_multiplier=-1,
            )
            # ind10[p, src, tl, j] = 1 iff j == 5*src + tl  (n2 row-select:
            # text rows 0-4, vis rows 5-9 of one [10, B] PSUM tile)
            ind10 = cpool.tile([128, 2, G, 2 * G], BF16, tag="ind10")
            nc.gpsimd.memset(ind10[:], 0.0)
            nc.gpsimd.affine_select(
                out=ind10[:],
                in_=ind10[:],
                compare_op=mybir.AluOpType.not_equal,
                fill=1.0,
                base=0,
                pattern=[[-G, 2], [-1, G], [1, 2 * G]],
                channel_multiplier=0,
            )
            # sel10[p, tl, d] = 1 iff p == 5 + tl (r_v row broadcast selector)
            sel10 = cpool.tile([2 * G, G, 128], BF16, tag="sel10")
            nc.gpsimd.memset(sel10[:], 0.0)
            nc.gpsimd.affine_select(
                out=sel10[:],
                in_=sel10[:],
                compare_op=mybir.AluOpType.not_equal,
                fill=1.0,
                base=-G,
                pattern=[[-1, G], [0, 128]],
                channel_multiplier=1,
            )
            ones64 = cpool.tile([ROWS, 1], F32, tag="ones64")
            nc.gpsimd.memset(ones64[:], 1.0)
            eps10 = cpool.tile([2 * G, 1], F32, tag="eps10")
            nc.gpsimd.memset(eps10[:], 1e-12)
            dmask = cpool.tile([ROWS, B], F32, tag="dmask")
            nc.sync.dma_start(dmask[:], dmask_in.ap())

            if _PRELOAD_SET:
                nc.scalar.add_instruction(
                    mybir.InstLoadActFuncSet(
                        name=nc.get_next_instruction_name(),
                        ins=[],
                        outs=[],
                        act_func_set_id=NL_EXP_SET_ID,
                    )
                )

            # ---- persistent SBUF ----
            textT = bigpool.tile([128, TLOC, 2, B], BF16, tag="textT")
            visT = bigpool.tile([128, TLOC, 2, B], BF16, tag="visT")
            rt_sb = bigpool.tile([128, NB, TLOC], F32, tag="rt")

            # ================= phase A: load + norms + normalize =========
            for g in range(NG):
                tg = ds(g * G, G)
                nc.gpsimd.dma_start(textT[:, tg, :, :], text_v[:, tg, :, :])
                nc.gpsimd.dma_start(visT[:, tg, :, :], vis_v[:, tg, :, :])

                n2_ps = ps_n2.tile([2 * G, B], F32, tag="n2")
                k = 0
                for src, xT in ((0, textT), (1, visT)):
                    sq = sqpool.tile([128, G, 2, B], BF16, tag=f"sq{src}")
                    nc.vector.tensor_mul(sq[:], xT[:, tg, :, :], xT[:, tg, :, :])
                    # pre-sum the two d-halves so n2 needs one matmul per t
                    sqh = sqpool.tile([128, G, B], BF16, tag=f"sqh{src}")
                    nc.vector.tensor_add(
                        sqh[:], sq[:, :, 0, :], sq[:, :, 1, :]
                    )
                    for tl in range(G):
                        nc.tensor.matmul(
                            n2_ps[:],
                            ind10[:, src, tl, :],
                            sqh[:, tl, :],
                            start=(k == 0),
                            stop=(k == 2 * G - 1),
                        )
                        k += 1
                # r rows = exp(-0.5 * ln(n2 + eps)):  [10, 512]
                scr = rrpool.tile([2 * G, B], F32, tag="scr")
                nc.scalar.activation(
                    scr[:], n2_ps[:],
                    mybir.ActivationFunctionType.Ln,
                    bias=eps10[:],
                )
                r = rrpool.tile([2 * G, B], BF16, tag="r")
                nc.scalar.activation(
                    r[:], scr[:],
                    mybir.ActivationFunctionType.Exp,
                    scale=-0.5,
                )

                # r_t -> [b-part, t] layout via PE mini-transposes
                for i in range(NB):
                    rtT_ps = ps_misc.tile([128, G], BF16, tag="rtT")
                    nc.tensor.transpose(
                        rtT_ps[:], r[0:G, ts(i, 128)], ident[0:G, 0:G]
                    )
                    nc.vector.tensor_copy(rt_sb[:, i, tg], rtT_ps[:])

                # normalize visT columns: rvb = ones (x) r_v[t]; visT *= rvb
                for tl in range(G):
                    tt = g * G + tl
                    rvb_ps = ps_misc.tile([128, B], F32, tag="rvb")
                    nc.tensor.matmul(
                        rvb_ps[:],
                        sel10[:, tl, :],
                        r[:],
                        start=True, stop=True,
                    )
                    rvb = rrpool.tile([128, B], BF16, tag="rvb_sb")
                    nc.scalar.copy(rvb[:], rvb_ps[:])
                    for h in range(2):
                        nc.vector.tensor_mul(
                            visT[:, tt, h, :], visT[:, tt, h, :], rvb[:]
                        )

            # ================= phase B: dots/exp/SV per b-tile ===========
            cc_outs = []
            for i in range(NB):
                S_ps = ps_acc.tile([128, B], F32, tag="Sps")
                V_ps = ps_acc.tile([128, B], F32, tag="Vps")
                pend = None
                for t in range(TLOC):
                    j = t % 2
                    if j == 0:
                        e_p = epool.tile([128, 2, B], F8, tag="e")
                        tmp_p = epool.tile([128, 2, B], F8, tag="tmp")
                    dots = ps_dots.tile([128, B], F32, tag="dots")
                    nc.tensor.matmul(
                        dots[:], textT[:, t, 0, ts(i, 128)], visT[:, t, 0, :],
                        start=True, stop=False,
                    )
                    nc.tensor.matmul(
                        dots[:], textT[:, t, 1, ts(i, 128)], visT[:, t, 1, :],
                        start=False, stop=True,
                    )
                    nc.scalar.activation(
                        e_p[:, j, :], dots[:],
                        mybir.ActivationFunctionType.Exp,
                        scale=rt_sb[:, i, ds(t, 1)],
                    )
                    nc.vector.scalar_tensor_tensor(
                        out=tmp_p[:, j, :],
                        in0=dots[:],
                        scalar=rt_sb[:, i, ds(t, 1)],
                        in1=e_p[:, j, :],
                        op0=mybir.AluOpType.mult,
                        op1=mybir.AluOpType.mult,
                    )
                    # S/V accumulation: fp8 DoubleRow per t-pair, emitted one
                    # pair LATE so the in-order PE never waits on fresh DVE
                    # output; plain fp8 matmul for the odd leftover slice
                    if j == 1:
                        if pend is not None:
                            pe, pt, pstart = pend
                            nc.tensor.matmul(
                                S_ps[:], identp[:], pe[:],
                                start=pstart, stop=False, perf_mode=DR,
                            )
                            nc.tensor.matmul(
                                V_ps[:], identp[:], pt[:],
                                start=pstart, stop=False, perf_mode=DR,
                            )
                        pend = (e_p, tmp_p, t == 1)
                    elif t == TLOC - 1:
                        pe, pt, pstart = pend
                        nc.tensor.matmul(
                            S_ps[:], identp[:], pe[:],
                            start=pstart, stop=False, perf_mode=DR,
                        )
                        nc.tensor.matmul(
                            V_ps[:], identp[:], pt[:],
                            start=pstart, stop=False, perf_mode=DR,
                        )
                        nc.tensor.matmul(
                            S_ps[:], identp[:, 0, :], e_p[:, 0, :],
                            start=False, stop=True,
                        )
                        nc.tensor.matmul(
                            V_ps[:], identp[:, 0, :], tmp_p[:, 0, :],
                            start=False, stop=True,
                        )
                # flush; ReduceScatter per b-tile (4 pipelined chunks)
                S_sb = svpool.tile([128, B], F32, tag="Ssb")
                V_sb = svpool.tile([128, B], F32, tag="Vsb")
                nc.vector.tensor_copy(S_sb[:], S_ps[:])
                nc.vector.tensor_copy(V_sb[:], V_ps[:])
                cc_in1 = dpool.tile([128, 2, B], F32, tag=f"cc_in{i}")
                nc.sync.dma_start(cc_in1[:, 0, :], S_sb[:])
                nc.sync.dma_start(cc_in1[:, 1, :], V_sb[:])
                cc_out = dpool.tile([RPC, 2, B], F32, tag=f"cc_out{i}")
                nc.gpsimd.collective_compute(
                    "ReduceScatter",
                    mybir.AluOpType.add,
                    replica_groups=[list(range(NCORES))],
                    ins=[cc_in1[:].opt()],
                    outs=[cc_out[:].opt()],
                )
                cc_outs.append(cc_out)

            # ================= tail: local loss on 64 owned rows =========
            # rows layout: chunk i at partitions 16i..16i+15
            rows = finpool.tile([ROWS, 2, B], F32, tag="rows")
            for i in range(NB):
                nc.sync.dma_start(rows[ds(RPC * i, RPC), :, :], cc_outs[i][:])
            Sl = rows[:, 0, :]
            Vl = rows[:, 1, :]
            # pad correction: each global pad slice added exp(0)=1 to S
            nc.vector.tensor_scalar_add(Sl, Sl, float(-NPAD))
            # sim = V / S  (S > 0)
            scr = finpool.tile([ROWS, B], F32, tag="scr")
            nc.scalar.activation(scr[:], Sl, mybir.ActivationFunctionType.Ln)
            nc.scalar.activation(
                scr[:], scr[:], mybir.ActivationFunctionType.Exp, scale=-1.0
            )
            sim = finpool.tile([ROWS, B], F32, tag="sim")
            nc.vector.tensor_mul(sim[:], Vl, scr[:])
            # diag partial (per-partition): sum(sim * dmask) over free dim
            diag_p = finpool.tile([ROWS, 1], F32, tag="diagp")
            nc.vector.scalar_tensor_tensor(
                out=scr[:],
                in0=sim[:],
                scalar=1.0,
                in1=dmask[:],
                op0=mybir.AluOpType.mult,
                op1=mybir.AluOpType.mult,
                accum_out=diag_p[:],
            )
            # e2 = exp(sim/TEMP) with fused rowsum
            e2 = finpool.tile([ROWS, B], F32, tag="e2")
            rowsum = finpool.tile([ROWS, 1], F32, tag="rowsum")
            nc.scalar.activation(
                e2[:], sim[:],
                mybir.ActivationFunctionType.Exp,
                scale=1.0 / TEMP,
                accum_out=rowsum[:],
            )
            lse_row = finpool.tile([ROWS, 1], F32, tag="lserow")
            nc.scalar.activation(
                lse_row[:], rowsum[:], mybir.ActivationFunctionType.Ln
            )
            # partition reductions via ones-matmuls: [1,512] colsum, [1,2] scalars
            col_ps = ps_misc.tile([1, B], F32, tag="rvb")
            nc.tensor.matmul(col_ps[:], ones64[:], e2[:], start=True, stop=True)
            red_pack = finpool.tile([ROWS, 2], F32, tag="redpack")
            nc.vector.tensor_copy(red_pack[:, 0:1], lse_row[:])
            nc.vector.tensor_copy(red_pack[:, 1:2], diag_p[:])
            red_ps = ps_misc.tile([1, 2], F32, tag="rtT")
            nc.tensor.matmul(red_ps[:], ones64[:], red_pack[:], start=True, stop=True)
            # pack [1, 514] = colsum | rowlse_sum | diag_sum
            pack = finpool.tile([1, B + 2], F32, tag="pack")
            nc.vector.tensor_copy(pack[:, 0:B], col_ps[:])
            nc.vector.tensor_copy(pack[:, B : B + 2], red_ps[:])
            cc2_in = dpool.tile([1, B + 2], F32, tag="cc2_in")
            cc2_out = dpool.tile([1, B + 2], F32, tag="cc2_out", addr_space="Shared")
            nc.sync.dma_start(cc2_in[:], pack[:])
            nc.gpsimd.collective_compute(
                "AllReduce",
                mybir.AluOpType.add,
                replica_groups=[list(range(NCORES))],
                ins=[cc2_in[:].opt()],
                outs=[cc2_out[:].opt()],
            )
            packr = finpool.tile([1, B + 2], F32, tag="packr")
            nc.sync.dma_start(packr[:], cc2_out[:])
            # loss = 0.5*(rowlse_sum + sum_c ln(colsum))/B - diag/(B*TEMP)
            lse_col = finpool.tile([1, B], F32, tag="lsecol")
            nc.scalar.activation(
                lse_col[:], packr[:, 0:B], mybir.ActivationFunctionType.Ln
            )
            csum = finpool.tile([1, 1], F32, tag="csum")
            nc.vector.reduce_sum(csum[:], lse_col[:], axis=mybir.AxisListType.X)
            t_a = finpool.tile([1, 1], F32, tag="ta")
            nc.vector.tensor_add(t_a[:], packr[:, B : B + 1], csum[:])
            nc.vector.tensor_scalar_mul(t_a[:], t_a[:], 0.5 / B)
            t_b = finpool.tile([1, 1], F32, tag="tb")
            nc.vector.tensor_scalar_mul(
                t_b[:], packr[:, B + 1 : B + 2], 1.0 / (B * TEMP)
            )
            loss_t = finpool.tile([1, 1], F32, tag="loss")
            nc.vector.tensor_sub(loss_t[:], t_a[:], t_b[:])
            nc.sync.dma_start(out.ap(), loss_t[:])

    nc.compile()
    return nc


_CACHE = {}


def _get_program():
    if "nc" not in _CACHE:
        _CACHE["nc"] = _build_program()
    return _CACHE["nc"]


def _stage(text: np.ndarray, vis: np.ndarray):
    """Host-side layout staging: pad T to 200, transpose to [t, d, b],
    split d into halves, slice per core. Zero FLOPs, pure data movement."""
    tt = np.zeros((TPAD, D, B), np.float32)
    vv = np.zeros((TPAD, D, B), np.float32)
    tt[:T] = text.transpose(1, 2, 0)
    vv[:T] = vis.transpose(1, 2, 0)
    tt = tt.reshape(TPAD, 2, 128, B)
    vv = vv.reshape(TPAD, 2, 128, B)
    in_maps = []
    for k in range(NCORES):
        sl = slice(k * TLOC, (k + 1) * TLOC)
        # rows owned after per-i RS: local p -> i = p//16, r = p%16;
        # global row = 128*i + 16*k + r
        p = np.arange(ROWS)
        rows = 128 * (p // RPC) + RPC * k + (p % RPC)
        dm = np.zeros((ROWS, B), np.float32)
        dm[np.arange(ROWS), rows] = 1.0
        in_maps.append(
            {
                "text": np.ascontiguousarray(tt[sl]),
                "vis": np.ascontiguousarray(vv[sl]),
                "dmask": dm,
            }
        )
    return in_maps


def kernel(text_tokens: np.ndarray, visual_tokens: np.ndarray) -> np.ndarray:
    text = np.asarray(text_tokens, dtype=np.float32)
    vis = np.asarray(visual_tokens, dtype=np.float32)
    assert text.shape == (B, T, D) and vis.shape == (B, T, D)
    in_maps = _stage(text, vis)
    nc = _get_program()
    res = run_bass_kernel_spmd(nc, in_maps, core_ids=list(range(NCORES)))
    loss = np.float32(res.results[0]["out"].reshape(-1)[0])
    return np.asarray(loss, dtype=np.float32).reshape(())
